# revision 3
# baseline (speedup 1.0000x reference)
"""BlockSparseLinear on 8 TRN2 NeuronCores — block-sparse PE-tiled kernel.

Computes out = x @ W_dense.T + bias where W_dense is [4096, 4096] assembled
from 8192 nonzero 32x32 blocks (50% density).

Strategy (v2, sparse):
  - Pure 8-way token sharding: each core gets 512 tokens and computes ALL
    4096 output features using ALL 8192 nonzero blocks.  The sparsity
    pattern (block_ids) is identical across cores, so all cores run the
    same SPMD program; only the x input differs.
  - The PE array is addressed as 16 independent 32x32 sub-arrays via
    tile_position.  Only NONZERO blocks are streamed: each block (r, c)
    becomes one 32x32x512 matmul at sub-array (i=c%4, j=j_r), halving PE
    work vs the dense kernel.
  - PSUM hazard discipline: block-row r is assigned a fixed column group
    j_r; phase-i partials (i = c%4) of row r accumulate in psum bank
    4p+i (p = quad parity), so concurrent row-tiles never write the same
    bank.  Within one (bank, 32-partition slice) all matmuls come from a
    single sub-array, so they serialize in hardware order.
  - Rows are processed in 32 "quads" of 4 rows (one per column group j).
    When a quad's 16 sub-chains finish, DVE combines the 4 partial banks
    + bias into an SBUF tile which is DMA'd out.
  - x is kept in its natural compact layout [128, 32 m, 512 t] bf16
    (k-block c lives at partitions 32*(c%4)); weights are streamed as
    prepacked bf16 chunks on the SP ring.
"""

import os
from itertools import permutations

import numpy as np
from ml_dtypes import bfloat16

import concourse.mybir as mybir
import concourse.tile as tile
from concourse import bacc
from concourse.bass_utils import run_bass_kernel_spmd

BLOCK = 32
IN_FEATURES = 4096
OUT_FEATURES = 4096
N_TOKENS = 4096
IB = IN_FEATURES // BLOCK  # 128 block-cols
OB = OUT_FEATURES // BLOCK  # 128 block-rows

N_CORES = 8
TSH = N_TOKENS // N_CORES  # 512 tokens per core
NFREE = 512
P = 128

NQUADS = 32  # 128 rows / 4 per quad
CB = 32  # blocks per sub-array per weight chunk
X_CHUNKS = 8  # x DMA chunks (32 m's split into 8 chunks of 4)

LAST_EXEC_NS = None
LAST_RESULT = None


def _install_axon_ntff_hook():
    try:
        from antenv.axon_hooks import get_axon_ntff_profile_hook

        return get_axon_ntff_profile_hook() is not None
    except ImportError:
        pass
    try:
        import sys
        import types

        import antenv
        import trn_agent_boot.trn_boot as tb

        hook = tb._ntff_profile_via_ctypes("/opt/axon/libaxon_pjrt.so")
        if hook is None:
            return False
        mod = types.ModuleType("antenv.axon_hooks")
        mod._hook = hook
        mod.get_axon_ntff_profile_hook = lambda: mod._hook
        mod.set_axon_ntff_profile_hook = lambda h: setattr(mod, "_hook", h)
        sys.modules["antenv.axon_hooks"] = mod
        antenv.axon_hooks = mod

        import concourse.bass_utils as bu

        bu.upload_artifacts = lambda tmpdir: str(tmpdir)
        return True
    except Exception:
        return False


class _Entry:
    __slots__ = ("q", "m", "start", "stop", "bidx")

    def __init__(self, q, m, start, stop, bidx):
        self.q = q
        self.m = m
        self.start = start
        self.stop = stop
        self.bidx = bidx  # index into block_ids, or -1 for dummy


def _schedule(block_ids):
    """Build the static per-sub-array schedule from the sparsity pattern."""
    ids = np.asarray(block_ids, dtype=np.int64)
    r_all = ids // IB
    c_all = ids % IB

    # per-row block lists (c's ascending since ids sorted)
    row_cs = [c_all[r_all == r] for r in range(OB)]
    row_bs = [np.nonzero(r_all == r)[0] for r in range(OB)]
    deg = np.array([len(cs) for cs in row_cs])
    deg_iv = np.zeros((OB, 4), dtype=np.int64)
    for r in range(OB):
        for i in range(4):
            deg_iv[r, i] = int(np.sum(row_cs[r] % 4 == i))

    # quads: snake by total degree so rows in a quad have similar length
    order = np.argsort(-deg, kind="stable")
    quad_rows = [order[4 * q : 4 * q + 4] for q in range(NQUADS)]

    # per-quad j assignment balancing per-(i,j) totals
    tot = np.zeros((4, 4), dtype=np.int64)
    assign = np.zeros((NQUADS, 4), dtype=np.int64)  # [q, j] -> row
    for q in range(NQUADS):
        rows4 = quad_rows[q]
        d4 = deg_iv[rows4]  # [4 rows, 4 phases]
        best = None
        for perm in permutations(range(4)):
            t2 = tot.copy()
            for k in range(4):
                t2[:, perm[k]] += d4[k]
            score = (t2.max(), t2.std())
            if best is None or score < best[0]:
                best = (score, perm)
        perm = best[1]
        for k in range(4):
            assign[q, perm[k]] = rows4[k]
            tot[:, perm[k]] += d4[k]

    # queues per sub-array (i, j)
    queues = {}
    for i in range(4):
        for j in range(4):
            entries = []
            for q in range(NQUADS):
                r = int(assign[q, j])
                cs = row_cs[r]
                bs = row_bs[r]
                sel = np.nonzero(cs % 4 == i)[0]
                if len(sel) == 0:
                    entries.append(_Entry(q, 0, True, True, -1))
                else:
                    n = len(sel)
                    for k, si in enumerate(sel):
                        entries.append(
                            _Entry(
                                q,
                                int(cs[si]) // 4,
                                k == 0,
                                k == n - 1,
                                int(bs[si]),
                            )
                        )
            queues[(i, j)] = entries

    # pad all queues to equal length T (multiple of CB) with trailing dummies
    T0 = max(len(v) for v in queues.values())
    T = ((T0 + CB - 1) // CB) * CB
    for sa, entries in queues.items():
        if len(entries) < T:
            last = entries[-1]
            last.stop = False
            npad = T - len(entries)
            for k in range(npad):
                entries.append(_Entry(last.q, 0, False, k == npad - 1, -1))
    nch = T // CB
    return {
        "queues": queues,
        "assign": assign,
        "T": T,
        "NCH": nch,
        "n_blocks": len(ids),
    }


# Latin-square sub-array emission order: consecutive 4 have distinct i and j
SA_ORDER = [((k % 4), (k // 4 + k % 4) % 4) for k in range(16)]


def _build_bass(sched):
    T = sched["T"]
    nch = sched["NCH"]
    queues = sched["queues"]

    nc = bacc.Bacc(None, target_bir_lowering=False)

    x_d = nc.dram_tensor("xh", [P, 32, TSH], mybir.dt.bfloat16, kind="ExternalInput")
    w_d = nc.dram_tensor(
        "wst", [nch, P, CB * 4 * BLOCK], mybir.dt.bfloat16, kind="ExternalInput"
    )
    b_d = nc.dram_tensor("biasq", [P, NQUADS], mybir.dt.float32, kind="ExternalInput")
    o_d = nc.dram_tensor("out", [NQUADS, P, TSH], mybir.dt.float32, kind="ExternalOutput")

    with tile.TileContext(nc) as tc:
        with (
            tc.tile_pool(name="xpool", bufs=1) as xpool,
            tc.tile_pool(name="wpool", bufs=4) as wpool,
            tc.tile_pool(name="spool", bufs=2) as spool,
            tc.tile_pool(name="bpool", bufs=1) as bpool,
            tc.tile_pool(name="psum", bufs=2, space="PSUM") as ppool,
        ):
            bias_sb = bpool.tile([P, NQUADS], mybir.dt.float32)
            nc.scalar.dma_start(bias_sb[:], b_d[:])

            x_sb = xpool.tile([P, 32, TSH], mybir.dt.bfloat16, tag="x", name="x")
            mm = 32 // X_CHUNKS
            for xc in range(X_CHUNKS):
                eng = nc.scalar if xc % 2 == 0 else nc.sync
                eng.dma_start(
                    x_sb[:, xc * mm : (xc + 1) * mm, :],
                    x_d[:, xc * mm : (xc + 1) * mm, :],
                )

            w_tiles = {}
            W_PREFETCH = 3

            def issue_w(ch):
                if ch < nch and ch not in w_tiles:
                    w_sb = wpool.tile(
                        [P, CB * 4 * BLOCK], mybir.dt.bfloat16, tag="w", name="w"
                    )
                    nc.sync.dma_start(w_sb[:], w_d[ch])
                    w_tiles[ch] = w_sb

            for ch in range(W_PREFETCH + 1):
                issue_w(ch)

            ps_tiles = {}  # (q, i) -> psum tile
            quad_left = [16] * NQUADS

            for t in range(T):
                ch = t // CB
                if t % CB == 0:
                    issue_w(ch + W_PREFETCH)
                w_sb = w_tiles[ch]
                for i, j in SA_ORDER:
                    e = queues[(i, j)][t]
                    key = (e.q, i)
                    if key not in ps_tiles:
                        ps_tiles[key] = ppool.tile(
                            [P, NFREE], mybir.dt.float32, tag=f"ps{i}", name="ps"
                        )
                    psum_t = ps_tiles[key]
                    col0 = ((t % CB) * 4 + j) * BLOCK
                    nc.tensor.matmul(
                        psum_t[32 * j : 32 * j + 32, :],
                        lhsT=w_sb[32 * i : 32 * i + 32, col0 : col0 + BLOCK],
                        rhs=x_sb[32 * i : 32 * i + 32, e.m, :],
                        start=e.start,
                        stop=e.stop,
                        tile_position=(32 * i, 32 * j),
                    )
                    if e.stop:
                        q = e.q
                        quad_left[q] -= 1
                        if quad_left[q] == 0:
                            # DVE may read at most one PSUM input per op:
                            # chain bank0+bias -> +bank1 -> +bank2 -> +bank3
                            pt = [ps_tiles.pop((q, ii)) for ii in range(4)]
                            s1 = spool.tile([P, NFREE], mybir.dt.float32, tag="s1")
                            nc.vector.tensor_tensor(
                                s1[:],
                                pt[0][:],
                                bias_sb[:, q : q + 1].to_broadcast([P, NFREE]),
                                mybir.AluOpType.add,
                            )
                            s2 = spool.tile([P, NFREE], mybir.dt.float32, tag="s2")
                            nc.vector.tensor_tensor(
                                s2[:], pt[1][:], s1[:], mybir.AluOpType.add
                            )
                            s3 = spool.tile([P, NFREE], mybir.dt.float32, tag="s3")
                            nc.vector.tensor_tensor(
                                s3[:], pt[2][:], s2[:], mybir.AluOpType.add
                            )
                            so = spool.tile([P, NFREE], mybir.dt.float32, tag="so")
                            nc.vector.tensor_tensor(
                                so[:], pt[3][:], s3[:], mybir.AluOpType.add
                            )
                            nc.scalar.dma_start(o_d[q], so[:])

    nc.compile()
    return nc


def _prep_weights(weight_data, sched):
    """Pack nonzero blocks into the streamed chunk tensor (bf16)."""
    queues = sched["queues"]
    T = sched["T"]
    nch = sched["NCH"]
    ch_l, i_l, slot_l, b_l = [], [], [], []
    for (i, j), entries in queues.items():
        for t, e in enumerate(entries):
            if e.bidx >= 0:
                ch_l.append(t // CB)
                i_l.append(i)
                slot_l.append((t % CB) * 4 + j)
                b_l.append(e.bidx)
    ch_a = np.array(ch_l)
    i_a = np.array(i_l)
    slot_a = np.array(slot_l)
    b_a = np.array(b_l)

    wdT = np.ascontiguousarray(weight_data.transpose(0, 2, 1)).astype(bfloat16)
    wfull = np.zeros((nch, 4, CB * 4, BLOCK, BLOCK), dtype=bfloat16)
    wfull[ch_a, i_a, slot_a] = wdT[b_a]
    w_np = np.ascontiguousarray(
        wfull.transpose(0, 1, 3, 2, 4).reshape(nch, P, CB * 4 * BLOCK)
    )
    return w_np


def _prep_x(x_shard):
    """[TSH, 4096] f32 -> [128, 32, TSH] bf16 with k-block c at partitions 32*(c%4)."""
    a = np.ascontiguousarray(x_shard.T).reshape(IB, BLOCK, TSH)  # [c, q, t]
    b = a.reshape(32, 4, BLOCK, TSH).transpose(1, 2, 0, 3)  # [i, q, m, t]
    return np.ascontiguousarray(b.reshape(P, 32, TSH)).astype(bfloat16)


def _prep_bias(bias, sched):
    assign = sched["assign"]
    bias_np = np.zeros((P, NQUADS), dtype=np.float32)
    for q in range(NQUADS):
        for j in range(4):
            r = int(assign[q, j])
            bias_np[32 * j : 32 * j + 32, q] = bias[32 * r : 32 * r + 32]
    return bias_np


def _assemble_out(o_cores, sched):
    """[per-core [NQUADS, 128, TSH] f32] -> [N_TOKENS, OUT_FEATURES]."""
    assign = sched["assign"]
    rflat = assign.reshape(-1)  # [q*4+j] -> row
    out = np.empty((N_TOKENS, OUT_FEATURES), dtype=np.float32)
    for core, o in enumerate(o_cores):
        o4 = o.reshape(NQUADS, 4, BLOCK, TSH)  # [q, j, oq, t]
        flat = o4.transpose(3, 0, 1, 2).reshape(TSH, OB, BLOCK)
        view = out[core * TSH : (core + 1) * TSH].reshape(TSH, OB, BLOCK)
        view[:, rflat, :] = flat
    return out


def _emulate_core(xh, w_np, bias_np, sched):
    """Numpy emulation of the device program for one core (for validation)."""
    queues = sched["queues"]
    T = sched["T"]
    o_d = np.zeros((NQUADS, P, TSH), dtype=np.float32)
    psum = {}
    for (i, j), entries in queues.items():
        for t, e in enumerate(entries):
            key = (e.q, i)
            if e.start or key not in psum:
                if key not in psum:
                    psum[key] = np.zeros((P, NFREE), dtype=np.float32)
                psum[key][32 * j : 32 * j + 32, :] = 0.0
            col0 = ((t % CB) * 4 + j) * BLOCK
            lhsT = w_np[t // CB, 32 * i : 32 * i + 32, col0 : col0 + BLOCK].astype(
                np.float32
            )
            rhs = xh[32 * i : 32 * i + 32, e.m, :].astype(np.float32)
            psum[key][32 * j : 32 * j + 32, :] += lhsT.T @ rhs
    for q in range(NQUADS):
        acc = sum(psum[(q, i)] for i in range(4))
        o_d[q] = acc + bias_np[:, q : q + 1]
    return o_d


def kernel(x, weight_data, bias, block_ids):
    x = np.ascontiguousarray(np.asarray(x, dtype=np.float32))
    weight_data = np.asarray(weight_data, dtype=np.float32)
    bias = np.asarray(bias, dtype=np.float32)
    block_ids = np.asarray(block_ids)

    sched = _schedule(block_ids)
    w_np = _prep_weights(weight_data, sched)
    bias_np = _prep_bias(bias, sched)
    xhs = [_prep_x(x[c * TSH : (c + 1) * TSH]) for c in range(N_CORES)]

    if bool(int(os.environ.get("BSL_EMU", "0"))):
        o_cores = [_emulate_core(xh, w_np, bias_np, sched) for xh in xhs]
        return _assemble_out(o_cores, sched)

    in_maps = [
        {"xh": xhs[c], "wst": w_np, "biasq": bias_np} for c in range(N_CORES)
    ]

    nc = _build_bass(sched)
    trace = bool(int(os.environ.get("BSL_TRACE", "0")))
    if trace:
        trace = _install_axon_ntff_hook()
    kwargs = {}
    if trace:
        tdir = os.environ.get("BSL_TRACE_DIR")
        if tdir:
            os.makedirs(tdir, exist_ok=True)
            kwargs["tmpdir"] = tdir
        kwargs["trace_cores"] = list(range(N_CORES))
    res = run_bass_kernel_spmd(
        nc,
        in_maps,
        core_ids=list(range(N_CORES)),
        trace=trace,
        **kwargs,
    )

    global LAST_EXEC_NS, LAST_RESULT
    LAST_EXEC_NS = res.exec_time_ns
    LAST_RESULT = res

    o_cores = [res.results[c]["out"] for c in range(N_CORES)]
    return _assemble_out(o_cores, sched)


# revision 4
# speedup vs baseline: 1.4903x; 1.4903x over previous
"""BlockSparseLinear on 8 TRN2 NeuronCores — block-sparse PE-tiled kernel (v3).

Computes out = x @ W_dense.T + bias where W_dense is [4096, 4096] assembled
from 8192 nonzero 32x32 blocks (50% density).

Strategy:
  - Pure 8-way token sharding: each core gets 512 tokens, all 8192 blocks.
    The sparsity pattern is shared, so all cores run one SPMD program.
  - Only nonzero blocks are computed, via PE sub-array tiling.  The v2
    lesson: every matmul pays ~34ns on the serial weight-load path
    (LDWEIGHTS streams its 32 columns at 1.2 GHz) regardless of stationary
    height.  So blocks are packed into the TALLEST stationaries possible:
    an x "slot" m holds 4 k-blocks (bands i=0..3 at partitions 32i); a row
    with >=2 blocks in one slot gets a single [128k, 32o] stationary
    (QUAD, absent bands zeroed, one LDW for up to 4 blocks).  Lone blocks
    stay [32k, 32o] SINGLEs.  A global knob balances total instruction
    issue (~34ns each) against PE stream occupancy (~15ns per block-slot).
  - Rows are processed in 32 windows of 4 rows (one per column strip j).
    PSUM discipline: tile -> bank 4p + lowest-band (p = window parity);
    same-(bank,slice) writers always share a sub-array path so hardware
    FIFO serializes them; different slices of one bank may be written
    concurrently (v2-proven).  Dummy zero-weight singles cover (row, band)
    groups that would otherwise leave a psum slice unstarted.
  - Per window, quads are emitted first, then singles (fewer PE tiling
    mode switches), each round-robin across strips/bands.
  - DVE combines the 4 partial banks + bias per window -> DMA out.
"""

import os
from itertools import permutations

import numpy as np
from ml_dtypes import bfloat16

import concourse.mybir as mybir
import concourse.tile as tile
from concourse import bacc
from concourse.bass_utils import run_bass_kernel_spmd

BLOCK = 32
IN_FEATURES = 4096
OUT_FEATURES = 4096
N_TOKENS = 4096
IB = IN_FEATURES // BLOCK  # 128 block-cols
OB = OUT_FEATURES // BLOCK  # 128 block-rows

N_CORES = 8
TSH = N_TOKENS // N_CORES  # 512 tokens per core
NFREE = 512
P = 128

NWIN = 32  # windows of 4 rows
CHUNK_SLOTS = 64  # 32-col weight slots per DMA chunk
X_CHUNKS = 8

ISSUE_NS = 34.0  # measured per-instruction issue cost (LDW+MM pair)
UNIT_NS = 15.0  # per block-slot stream cost (512 cyc / 16 sub-arrays)

LAST_EXEC_NS = None
LAST_RESULT = None


def _install_axon_ntff_hook():
    try:
        from antenv.axon_hooks import get_axon_ntff_profile_hook

        return get_axon_ntff_profile_hook() is not None
    except ImportError:
        pass
    try:
        import sys
        import types

        import antenv
        import trn_agent_boot.trn_boot as tb

        hook = tb._ntff_profile_via_ctypes("/opt/axon/libaxon_pjrt.so")
        if hook is None:
            return False
        mod = types.ModuleType("antenv.axon_hooks")
        mod._hook = hook
        mod.get_axon_ntff_profile_hook = lambda: mod._hook
        mod.set_axon_ntff_profile_hook = lambda h: setattr(mod, "_hook", h)
        sys.modules["antenv.axon_hooks"] = mod
        antenv.axon_hooks = mod

        import concourse.bass_utils as bu

        bu.upload_artifacts = lambda tmpdir: str(tmpdir)
        return True
    except Exception:
        return False


class _Tile:
    """One PE instruction: a QUAD ([128,32] stationary) or SINGLE ([32,32])."""

    __slots__ = ("kind", "row", "m", "blocks", "j", "w", "slot", "start", "stop")

    def __init__(self, kind, row, m, blocks):
        self.kind = kind  # 'q' or 's'
        self.row = row
        self.m = m
        self.blocks = blocks  # list of (band, bidx); bidx -1 = zero dummy
        self.j = -1
        self.w = -1
        self.slot = -1
        self.start = False
        self.stop = False

    @property
    def lowband(self):
        return 0 if self.kind == "q" else self.blocks[0][0]


def _schedule(block_ids):
    ids = np.asarray(block_ids, dtype=np.int64)
    r_all = ids // IB
    c_all = ids % IB

    # per-row: dict m -> list of (band, bidx)
    row_slots = [dict() for _ in range(OB)]
    for b in range(len(ids)):
        r = int(r_all[b])
        c = int(c_all[b])
        row_slots[r].setdefault(c // 4, []).append((c % 4, b))

    # count k2 slots; balance quads-vs-singles for them
    k2_slots = []
    base_i = base_u = 0
    for r in range(OB):
        for m, blks in row_slots[r].items():
            k = len(blks)
            if k == 2:
                k2_slots.append((r, m))
            elif k == 1:
                base_i += 1
                base_u += 1
            else:
                base_i += 1
                base_u += 4
    K2 = len(k2_slots)
    best = None
    for x in range(K2 + 1):
        ins = base_i + (K2 - x) + 2 * x
        un = base_u + 4 * (K2 - x) + 2 * x
        t = max(ISSUE_NS * ins, UNIT_NS * un)
        if best is None or t < best[0]:
            best = (t, x)
    x_opt = best[1]
    # spread k2->single conversions round-robin over rows
    byrow = {}
    for r, m in k2_slots:
        byrow.setdefault(r, []).append(m)
    k2_single = set()
    cnt = 0
    while cnt < x_opt:
        prog = False
        for r, ms in byrow.items():
            if ms and cnt < x_opt:
                k2_single.add((r, ms.pop()))
                cnt += 1
                prog = True
        if not prog:
            break

    # build per-row tiles
    row_tiles = []
    for r in range(OB):
        tiles = []
        for m in sorted(row_slots[r]):
            blks = sorted(row_slots[r][m])
            k = len(blks)
            if k == 1 or (k == 2 and (r, m) in k2_single):
                for band, bidx in blks:
                    tiles.append(_Tile("s", r, m, [(band, bidx)]))
            else:
                tiles.append(_Tile("q", r, m, blks))
        # guarantee every band group has a writer (psum slice validity)
        havebands = set(t.lowband for t in tiles)
        if not any(t.kind == "q" for t in tiles) and 0 not in havebands:
            pass  # band 0 covered below by dummy single if needed
        for b in range(4):
            if b not in havebands:
                tiles.append(_Tile("s", r, 0, [(b, -1)]))
        row_tiles.append(tiles)

    units = np.array(
        [sum(4 if t.kind == "q" else 1 for t in tiles) for tiles in row_tiles]
    )

    # windows: snake by units, then per-window strip assignment balancing
    order = np.argsort(-units, kind="stable")
    win_rows = [order[4 * w : 4 * w + 4] for w in range(NWIN)]
    tot_u = np.zeros(4)
    tot_q = np.zeros(4)
    assign = np.zeros((NWIN, 4), dtype=np.int64)  # [w, j] -> row
    for w in range(NWIN):
        rows4 = win_rows[w]
        u4 = units[rows4].astype(float)
        q4 = np.array(
            [sum(1 for t in row_tiles[r] if t.kind == "q") for r in rows4],
            dtype=float,
        )
        best = None
        for perm in permutations(range(4)):
            tu = tot_u.copy()
            tq = tot_q.copy()
            for k in range(4):
                tu[perm[k]] += u4[k]
                tq[perm[k]] += q4[k]
            score = (tq.max() - tq.min(), tu.max() - tu.min())
            if best is None or score < best[0]:
                best = (score, perm)
        perm = best[1]
        for k in range(4):
            j = perm[k]
            r = int(rows4[k])
            assign[w, j] = r
            for t in row_tiles[r]:
                t.j = j
                t.w = w
            tot_u[j] += u4[k]
            tot_q[j] += q4[k]

    # emission order + slot assignment
    emit = []  # list over windows of instruction lists
    slot_base = 0
    for w in range(NWIN):
        wl = []
        strips = [[], [], [], []]
        for j in range(4):
            r = int(assign[w, j])
            strips[j] = row_tiles[r]
        # quads round-robin over strips
        qlists = [[t for t in strips[j] if t.kind == "q"] for j in range(4)]
        slists = [[t for t in strips[j] if t.kind == "s"] for j in range(4)]
        # band-interleave each strip's singles
        for j in range(4):
            byband = [[], [], [], []]
            for t in slists[j]:
                byband[t.lowband].append(t)
            inter = []
            k = 0
            while any(byband):
                b = k % 4
                if byband[b]:
                    inter.append(byband[b].pop(0))
                k += 1
                if k > 10000:
                    break
            slists[j] = inter
        qi = [0] * 4
        while True:
            prog = False
            for j in range(4):
                if qi[j] < len(qlists[j]):
                    t = qlists[j][qi[j]]
                    t.slot = slot_base
                    slot_base += 1
                    wl.append(t)
                    qi[j] += 1
                    prog = True
            if not prog:
                break
        lane = [0, 0, 0, 0]  # per-band lane counter for singles slots
        si = [0] * 4
        sl = []
        while True:
            prog = False
            for j in range(4):
                if si[j] < len(slists[j]):
                    t = slists[j][si[j]]
                    b = t.lowband
                    t.slot = slot_base + lane[b]
                    lane[b] += 1
                    sl.append(t)
                    si[j] += 1
                    prog = True
            if not prog:
                break
        slot_base += max(lane) if any(lane) else 0
        wl.extend(sl)
        emit.append(wl)

    # start/stop flags per (row, lowband) group, in emission order
    groups = {}
    for wl in emit:
        for t in wl:
            groups.setdefault((t.row, t.lowband), []).append(t)
    for key, ts in groups.items():
        ts[0].start = True
        ts[-1].stop = True

    n_slots = slot_base
    nch = (n_slots + CHUNK_SLOTS - 1) // CHUNK_SLOTS
    return {
        "emit": emit,
        "assign": assign,
        "NCH": nch,
        "n_slots": n_slots,
        "n_instr": sum(len(wl) for wl in emit),
    }


def _build_bass(sched):
    nch = sched["NCH"]
    emit = sched["emit"]

    nc = bacc.Bacc(None, target_bir_lowering=False)

    x_d = nc.dram_tensor("xh", [P, 32, TSH], mybir.dt.bfloat16, kind="ExternalInput")
    w_d = nc.dram_tensor(
        "wst", [nch, P, CHUNK_SLOTS * BLOCK], mybir.dt.bfloat16, kind="ExternalInput"
    )
    b_d = nc.dram_tensor("biasq", [P, NWIN], mybir.dt.float32, kind="ExternalInput")
    o_d = nc.dram_tensor("out", [NWIN, P, TSH], mybir.dt.float32, kind="ExternalOutput")

    with tile.TileContext(nc) as tc:
        with (
            tc.tile_pool(name="xpool", bufs=1) as xpool,
            tc.tile_pool(name="wpool", bufs=4) as wpool,
            tc.tile_pool(name="spool", bufs=2) as spool,
            tc.tile_pool(name="bpool", bufs=1) as bpool,
            tc.tile_pool(name="psum", bufs=2, space="PSUM") as ppool,
        ):
            bias_sb = bpool.tile([P, NWIN], mybir.dt.float32)
            nc.scalar.dma_start(bias_sb[:], b_d[:])

            x_sb = xpool.tile([P, 32, TSH], mybir.dt.bfloat16, tag="x", name="x")
            mm = 32 // X_CHUNKS
            for xc in range(X_CHUNKS):
                eng = nc.scalar if xc % 2 == 0 else nc.sync
                eng.dma_start(
                    x_sb[:, xc * mm : (xc + 1) * mm, :],
                    x_d[:, xc * mm : (xc + 1) * mm, :],
                )

            w_tiles = {}
            W_PREFETCH = 3

            def issue_w(ch):
                if ch < nch and ch not in w_tiles:
                    w_sb = wpool.tile(
                        [P, CHUNK_SLOTS * BLOCK], mybir.dt.bfloat16, tag="w", name="w"
                    )
                    nc.sync.dma_start(w_sb[:], w_d[ch])
                    w_tiles[ch] = w_sb

            for ch in range(W_PREFETCH + 1):
                issue_w(ch)

            ps_tiles = {}
            win_left = [16] * NWIN
            issued_ch = 0

            for w in range(NWIN):
                for t in emit[w]:
                    ch = t.slot // CHUNK_SLOTS
                    while issued_ch < ch:
                        issued_ch += 1
                        issue_w(issued_ch + W_PREFETCH)
                    w_sb = w_tiles[ch]
                    col0 = (t.slot % CHUNK_SLOTS) * BLOCK
                    key = (t.w, t.lowband)
                    if key not in ps_tiles:
                        ps_tiles[key] = ppool.tile(
                            [P, NFREE],
                            mybir.dt.float32,
                            tag=f"ps{t.lowband}",
                            name="ps",
                        )
                    psum_t = ps_tiles[key]
                    j = t.j
                    if t.kind == "q":
                        nc.tensor.matmul(
                            psum_t[32 * j : 32 * j + 32, :],
                            lhsT=w_sb[:, col0 : col0 + BLOCK],
                            rhs=x_sb[:, t.m, :],
                            start=t.start,
                            stop=t.stop,
                            tile_position=(0, 32 * j),
                        )
                    else:
                        b = t.lowband
                        nc.tensor.matmul(
                            psum_t[32 * j : 32 * j + 32, :],
                            lhsT=w_sb[32 * b : 32 * b + 32, col0 : col0 + BLOCK],
                            rhs=x_sb[32 * b : 32 * b + 32, t.m, :],
                            start=t.start,
                            stop=t.stop,
                            tile_position=(32 * b, 32 * j),
                        )
                    if t.stop:
                        win_left[t.w] -= 1
                        if win_left[t.w] == 0:
                            q = t.w
                            pt = [ps_tiles.pop((q, ii)) for ii in range(4)]
                            s1 = spool.tile([P, NFREE], mybir.dt.float32, tag="s1")
                            nc.vector.tensor_tensor(
                                s1[:],
                                pt[0][:],
                                bias_sb[:, q : q + 1].to_broadcast([P, NFREE]),
                                mybir.AluOpType.add,
                            )
                            s2 = spool.tile([P, NFREE], mybir.dt.float32, tag="s2")
                            nc.vector.tensor_tensor(
                                s2[:], pt[1][:], s1[:], mybir.AluOpType.add
                            )
                            s3 = spool.tile([P, NFREE], mybir.dt.float32, tag="s3")
                            nc.vector.tensor_tensor(
                                s3[:], pt[2][:], s2[:], mybir.AluOpType.add
                            )
                            so = spool.tile([P, NFREE], mybir.dt.float32, tag="so")
                            nc.vector.tensor_tensor(
                                so[:], pt[3][:], s3[:], mybir.AluOpType.add
                            )
                            nc.scalar.dma_start(o_d[q], so[:])

    nc.compile()
    return nc


def _prep_weights(weight_data, sched):
    nch = sched["NCH"]
    wdT = np.ascontiguousarray(weight_data.transpose(0, 2, 1)).astype(bfloat16)
    w_np = np.zeros((nch, P, CHUNK_SLOTS * BLOCK), dtype=bfloat16)
    for wl in sched["emit"]:
        for t in wl:
            ch = t.slot // CHUNK_SLOTS
            col0 = (t.slot % CHUNK_SLOTS) * BLOCK
            for band, bidx in t.blocks:
                if bidx >= 0:
                    w_np[ch, 32 * band : 32 * band + 32, col0 : col0 + BLOCK] = wdT[
                        bidx
                    ]
    return w_np


def _prep_x(x_shard):
    a = np.ascontiguousarray(x_shard.T).reshape(IB, BLOCK, TSH)  # [c, q, t]
    b = a.reshape(32, 4, BLOCK, TSH).transpose(1, 2, 0, 3)  # [i, q, m, t]
    return np.ascontiguousarray(b.reshape(P, 32, TSH)).astype(bfloat16)


def _prep_bias(bias, sched):
    assign = sched["assign"]
    bias_np = np.zeros((P, NWIN), dtype=np.float32)
    for q in range(NWIN):
        for j in range(4):
            r = int(assign[q, j])
            bias_np[32 * j : 32 * j + 32, q] = bias[32 * r : 32 * r + 32]
    return bias_np


def _assemble_out(o_cores, sched):
    assign = sched["assign"]
    rflat = assign.reshape(-1)
    out = np.empty((N_TOKENS, OUT_FEATURES), dtype=np.float32)
    for core, o in enumerate(o_cores):
        o4 = o.reshape(NWIN, 4, BLOCK, TSH)
        flat = o4.transpose(3, 0, 1, 2).reshape(TSH, OB, BLOCK)
        view = out[core * TSH : (core + 1) * TSH].reshape(TSH, OB, BLOCK)
        view[:, rflat, :] = flat
    return out


def _emulate_core(xh, w_np, bias_np, sched):
    o_d = np.zeros((NWIN, P, TSH), dtype=np.float32)
    psum = {}
    for wl in sched["emit"]:
        for t in wl:
            key = (t.w, t.lowband)
            if key not in psum:
                psum[key] = np.zeros((P, NFREE), dtype=np.float32)
            if t.start:
                psum[key][32 * t.j : 32 * t.j + 32, :] = 0.0
            ch = t.slot // CHUNK_SLOTS
            col0 = (t.slot % CHUNK_SLOTS) * BLOCK
            if t.kind == "q":
                lhsT = w_np[ch, :, col0 : col0 + BLOCK].astype(np.float32)
                rhs = xh[:, t.m, :].astype(np.float32)
            else:
                b = t.lowband
                lhsT = w_np[ch, 32 * b : 32 * b + 32, col0 : col0 + BLOCK].astype(
                    np.float32
                )
                rhs = xh[32 * b : 32 * b + 32, t.m, :].astype(np.float32)
            psum[key][32 * t.j : 32 * t.j + 32, :] += lhsT.T @ rhs
    for q in range(NWIN):
        acc = sum(psum[(q, i)] for i in range(4))
        o_d[q] = acc + bias_np[:, q : q + 1]
    return o_d


def kernel(x, weight_data, bias, block_ids):
    x = np.ascontiguousarray(np.asarray(x, dtype=np.float32))
    weight_data = np.asarray(weight_data, dtype=np.float32)
    bias = np.asarray(bias, dtype=np.float32)
    block_ids = np.asarray(block_ids)

    sched = _schedule(block_ids)
    w_np = _prep_weights(weight_data, sched)
    bias_np = _prep_bias(bias, sched)
    xhs = [_prep_x(x[c * TSH : (c + 1) * TSH]) for c in range(N_CORES)]

    if bool(int(os.environ.get("BSL_EMU", "0"))):
        o_cores = [_emulate_core(xh, w_np, bias_np, sched) for xh in xhs]
        return _assemble_out(o_cores, sched)

    in_maps = [{"xh": xhs[c], "wst": w_np, "biasq": bias_np} for c in range(N_CORES)]

    nc = _build_bass(sched)
    trace = bool(int(os.environ.get("BSL_TRACE", "0")))
    if trace:
        trace = _install_axon_ntff_hook()
    kwargs = {}
    if trace:
        tdir = os.environ.get("BSL_TRACE_DIR")
        if tdir:
            os.makedirs(tdir, exist_ok=True)
            kwargs["tmpdir"] = tdir
        kwargs["trace_cores"] = list(range(N_CORES))
    res = run_bass_kernel_spmd(
        nc,
        in_maps,
        core_ids=list(range(N_CORES)),
        trace=trace,
        **kwargs,
    )

    global LAST_EXEC_NS, LAST_RESULT
    LAST_EXEC_NS = res.exec_time_ns
    LAST_RESULT = res

    o_cores = [res.results[c]["out"] for c in range(N_CORES)]
    return _assemble_out(o_cores, sched)
